# revision 4
# baseline (speedup 1.0000x reference)
"""GCN critic kernel for 8 trn2 NeuronCores (Bass/Tile, SPMD).

Math (matches the jax reference):
  deg = indegree(dst) + 1 ; dis = deg^-1/2
  layer(x, W, b):  g = dis * (x @ W)            (rows pre-scaled)
                   agg[n] = sum_{e: dst=n} g[src_e]        (+ g[n] self term)
                   out = relu(dis * agg + b)
  pooled = mean_{batch} h2 ; z = [pooled|state|action] ; q = MLP(z)

Distribution: nodes are padded to 100352 slots = 8 cores x 98 blocks x 128.
Each core owns the aggregation for its 12544 dst slots. Edges are bucketed
by src range (4 buckets of 25088 rows, int16 dma_gather addressing) and
padded per (block, bucket) to a static segment length S. Per-edge messages
are fetched with dma_gather from a bf16 node table (allgathered between
layers) and segment-summed on the tensor engine via one-hot indicator
matmuls accumulating in PSUM. Graph pooling is an indicator matmul too;
the small MLP head runs replicated on every core after an AllReduce.

Host work is limited to integer index preprocessing (sort/pad/layout) and
data movement; all floating-point math runs on the NeuronCores.
"""
import os
import sys
import types
import numpy as np

sys.path.insert(0, "/opt/trn_rl_repo")

import ml_dtypes

# ---------------- problem constants (hardcoded per contract) ----------------
N = 100000
E = 1600000
G = 64
F = 128
NC = 8
NBLK = 98                 # blocks per core
PB = NBLK * 128           # 12544 slots per core
NTAB = PB * NC            # 100352 padded node slots
NBUCK = 4
BUCK = NTAB // NBUCK      # 25088 rows per gather bucket (< int16 max)
GRP = 14                  # gather groups per core
GBLK = NBLK // GRP        # 7 blocks per group

LAST_RESULT = None        # stash of the last BassKernelResults (for test.py)
_PROGRAM_CACHE = {}


def _install_ntff_hook():
    try:
        import antenv.axon_hooks  # noqa: F401
        return
    except ImportError:
        pass
    try:
        from trn_agent_boot.trn_boot import _ntff_profile_via_ctypes
        import antenv
        mod = types.ModuleType("antenv.axon_hooks")
        mod._hook = _ntff_profile_via_ctypes("/opt/axon/libaxon_pjrt.so")
        mod.set_axon_ntff_profile_hook = lambda h: setattr(mod, "_hook", h)
        mod.get_axon_ntff_profile_hook = lambda: mod._hook
        sys.modules["antenv.axon_hooks"] = mod
        antenv.axon_hooks = mod
    except Exception:
        pass


# ---------------- host-side integer preprocessing ----------------

def _preprocess(inputs):
    src = np.asarray(inputs["edge_index"][0], np.int64)
    dst = np.asarray(inputs["edge_index"][1], np.int64)
    batch = np.asarray(inputs["batch"], np.int64)
    x = np.asarray(inputs["x"], np.float32)

    blk = dst >> 7                      # global block id, 0..781
    bucket = src // BUCK                # 0..3
    key = blk * NBUCK + bucket
    counts = np.bincount(key, minlength=NC * NBLK * NBUCK)
    S = int(max(640, -(-int(counts.max()) // 128) * 128))
    L = GBLK * S                        # per (group, bucket) stream length
    TGK = L // 128                      # tiles per (group, bucket)

    order = np.argsort(key, kind="stable")
    starts = np.zeros(NC * NBLK * NBUCK, np.int64)
    starts[1:] = np.cumsum(counts)[:-1]
    rows = key[order]
    pos = np.arange(E, dtype=np.int64) - starts[rows]

    idx_pad = np.zeros((NC * NBLK * NBUCK, S), np.int16)
    ld_pad = np.full((NC * NBLK * NBUCK, S), 384.0, np.float32)
    idx_pad[rows, pos] = (src[order] - bucket[order] * BUCK).astype(np.int16)
    ld_pad[rows, pos] = (dst[order] & 127).astype(np.float32)

    idx_pad = idx_pad.reshape(NC, GRP, GBLK, NBUCK, S)
    ld_pad = ld_pad.reshape(NC, GRP, GBLK, NBUCK, S)

    # idx feed: [NC, GRP, 128, NBUCK*(L//16)] wrapped-16 (i%16, i//16), x8 replicated
    idx_s = idx_pad.transpose(0, 1, 3, 2, 4).reshape(NC, GRP, NBUCK, L)
    w = idx_s.reshape(NC, GRP, NBUCK, L // 16, 16).transpose(0, 1, 2, 4, 3)
    idx_feed = np.broadcast_to(
        w[:, :, None], (NC, GRP, 8, NBUCK, 16, L // 16)
    ).reshape(NC, GRP, 8 * 16, NBUCK * (L // 16))
    # -> [NC, GRP, 128, NBUCK*L/16] but partition-major needs (rep, 16) order:
    idx_feed = np.ascontiguousarray(
        np.broadcast_to(w[:, :, :, None], (NC, GRP, NBUCK, 8, 16, L // 16))
        .transpose(0, 1, 3, 4, 2, 5)
        .reshape(NC, GRP, 128, NBUCK * (L // 16))
    )

    # ld feed: [NC, GRP, 128, NBUCK*TGK]; [p, k*TGK+t] = ld of edge t*128+p
    ld_s = ld_pad.transpose(0, 1, 3, 2, 4).reshape(NC, GRP, NBUCK, TGK, 128)
    ld_feed = np.ascontiguousarray(
        ld_s.transpose(0, 1, 4, 2, 3).reshape(NC, GRP, 128, NBUCK * TGK)
    )

    # degree (with self loop) and batch ids, padded to slots, [128, NBLK] layout
    deg = np.ones(NTAB, np.float32)
    deg[:N] += np.bincount(dst, minlength=N).astype(np.float32)
    bat = np.full(NTAB, 384.0, np.float32)
    bat[:N] = batch.astype(np.float32)
    deg_feed = deg.reshape(NC, NBLK, 128).transpose(0, 2, 1)
    bat_feed = bat.reshape(NC, NBLK, 128).transpose(0, 2, 1)

    # transposed x shards [NC, 128, PB]
    x_pad = np.zeros((NTAB, F), np.float32)
    x_pad[:N] = x
    xt_feed = x_pad.reshape(NC, PB, F).transpose(0, 2, 1)

    return dict(
        S=S, L=L, TGK=TGK,
        idx_feed=idx_feed, ld_feed=ld_feed,
        deg_feed=np.ascontiguousarray(deg_feed),
        bat_feed=np.ascontiguousarray(bat_feed),
        xt_feed=np.ascontiguousarray(xt_feed),
    )


# ---------------- device program ----------------

def _build_program(S):
    import concourse.bacc as bacc
    import concourse.mybir as mybir
    import concourse.tile as tile

    f32 = mybir.dt.float32
    bf16 = mybir.dt.bfloat16
    i16 = mybir.dt.int16
    Alu = mybir.AluOpType
    Act = mybir.ActivationFunctionType

    L = GBLK * S
    TGK = L // 16          # idx columns per (group, bucket)
    TT = L // 128          # tiles per (group, bucket)
    TB = S // 128          # tiles per (block, bucket)

    nc = bacc.Bacc("TRN2", target_bir_lowering=False, debug=False, num_devices=NC)

    # inputs
    xt = nc.dram_tensor("xt", [128, PB], f32, kind="ExternalInput")
    deg = nc.dram_tensor("deg", [128, NBLK], f32, kind="ExternalInput")
    bat = nc.dram_tensor("bat", [128, NBLK], f32, kind="ExternalInput")
    idxs = nc.dram_tensor("idxs", [GRP, 128, NBUCK * TGK], i16, kind="ExternalInput")
    lds = nc.dram_tensor("lds", [GRP, 128, NBUCK * TT], f32, kind="ExternalInput")
    iota = nc.dram_tensor("iota", [128, 128], bf16, kind="ExternalInput")
    ident = nc.dram_tensor("ident", [128, 128], bf16, kind="ExternalInput")
    id64 = nc.dram_tensor("id64", [64, 64], f32, kind="ExternalInput")
    b1t = nc.dram_tensor("b1t", [128, 128], f32, kind="ExternalInput")
    b2t = nc.dram_tensor("b2t", [128, 128], f32, kind="ExternalInput")
    w1 = nc.dram_tensor("w1", [128, 128], f32, kind="ExternalInput")
    w2 = nc.dram_tensor("w2", [128, 128], f32, kind="ExternalInput")
    wf1 = nc.dram_tensor("wf1", [224, 256], f32, kind="ExternalInput")
    wf2 = nc.dram_tensor("wf2", [256, 256], f32, kind="ExternalInput")
    wo2 = nc.dram_tensor("wo2", [128, 2], f32, kind="ExternalInput")
    bf1 = nc.dram_tensor("bf1", [128, 2], f32, kind="ExternalInput")
    bf2 = nc.dram_tensor("bf2", [128, 2], f32, kind="ExternalInput")
    bo = nc.dram_tensor("bo", [1, 1], f32, kind="ExternalInput")
    st_t = nc.dram_tensor("st_t", [64, 64], f32, kind="ExternalInput")
    ac_t = nc.dram_tensor("ac_t", [32, 64], f32, kind="ExternalInput")
    q_out = nc.dram_tensor("q_out", [1, 64], f32, kind="ExternalOutput")

    with tile.TileContext(nc) as tc:
        with tc.tile_pool(name="const", bufs=1) as cp, \
             tc.tile_pool(name="dram", bufs=1, space="DRAM") as dram:
            # ---- constants ----
            iota_sb = cp.tile([128, 128], bf16)
            nc.sync.dma_start(out=iota_sb[:], in_=iota[:, :])
            ident_sb = cp.tile([128, 128], bf16)
            nc.sync.dma_start(out=ident_sb[:], in_=ident[:, :])
            id64_sb = cp.tile([64, 64], f32)
            nc.sync.dma_start(out=id64_sb[:], in_=id64[:, :])
            b1_sb = cp.tile([128, 128], f32)
            nc.sync.dma_start(out=b1_sb[:], in_=b1t[:, :])
            b2_sb = cp.tile([128, 128], f32)
            nc.sync.dma_start(out=b2_sb[:], in_=b2t[:, :])
            w1_f = cp.tile([128, 128], f32)
            nc.sync.dma_start(out=w1_f[:], in_=w1[:, :])
            w2_f = cp.tile([128, 128], f32)
            nc.sync.dma_start(out=w2_f[:], in_=w2[:, :])
            w1_sb = cp.tile([128, 128], bf16)
            nc.vector.tensor_copy(w1_sb[:], w1_f[:])
            w2_sb = cp.tile([128, 128], bf16)
            nc.vector.tensor_copy(w2_sb[:], w2_f[:])
            deg_sb = cp.tile([128, NBLK], f32)
            nc.sync.dma_start(out=deg_sb[:], in_=deg[:, :])
            bat_sb = cp.tile([128, NBLK], f32)
            nc.sync.dma_start(out=bat_sb[:], in_=bat[:, :])
            dis_sb = cp.tile([128, NBLK], f32)
            nc.vector.reciprocal(dis_sb[:], deg_sb[:])
            nc.scalar.activation(dis_sb[:], dis_sb[:], Act.Sqrt)

            g0_shard = dram.tile([PB, F], bf16)
            g1_shard = dram.tile([PB, F], bf16)
            g0_tab = dram.tile([NTAB, F], bf16)
            g1_tab = dram.tile([NTAB, F], bf16)

            # ---- phase A: g0 shard = dis * (x @ W1), allgather ----
            with tc.tile_pool(name="pha", bufs=1) as xp, \
                 tc.tile_pool(name="pha_w", bufs=3) as wp, \
                 tc.tile_pool(name="pha_ps", bufs=3, space="PSUM") as pp:
                xt_sb = xp.tile([128, PB], f32)
                nc.sync.dma_start(out=xt_sb[:], in_=xt[:, :])
                for b in range(NBLK):
                    xbf = wp.tile([128, 128], bf16, tag="xbf")
                    nc.vector.tensor_copy(xbf[:], xt_sb[:, b * 128:(b + 1) * 128])
                    h0 = pp.tile([128, 128], f32, tag="h0")
                    nc.tensor.matmul(h0[:], xbf[:], w1_sb[:], start=True, stop=True)
                    g0b = wp.tile([128, 128], bf16, tag="g0b")
                    nc.vector.tensor_scalar(
                        g0b[:], h0[:], dis_sb[:, b:b + 1], None, Alu.mult)
                    nc.sync.dma_start(
                        out=g0_shard[:][b * 128:(b + 1) * 128, :], in_=g0b[:])
            nc.gpsimd.collective_compute(
                "AllGather", Alu.bypass,
                replica_groups=[list(range(NC))],
                ins=[g0_shard[:].opt()], outs=[g0_tab[:].opt()])

            # ---- phases B (layer 1) and C (layer 2) ----
            def edge_layer(layer, g_tab, g_shard_self, out_shard, b_sb, pool_ps):
                with tc.tile_pool(name=f"l{layer}_st", bufs=2) as sp, \
                     tc.tile_pool(name=f"l{layer}_wk", bufs=3) as wp, \
                     tc.tile_pool(name=f"l{layer}_ps", bufs=2, space="PSUM") as pp, \
                     tc.tile_pool(name=f"l{layer}_pt", bufs=2, space="PSUM") as pt:
                        for g in range(GRP):
                            idx_sb = sp.tile([128, NBUCK * TGK], i16, tag="idx")
                            nc.sync.dma_start(out=idx_sb[:], in_=idxs[g, :, :])
                            ld_sb = sp.tile([128, NBUCK * TT], f32, tag="ld")
                            nc.sync.dma_start(out=ld_sb[:], in_=lds[g, :, :])
                            streams = []
                            for k in range(NBUCK):
                                stt = sp.tile([128, L], bf16, tag=f"gst{k}")
                                nc.gpsimd.dma_gather(
                                    stt[:].rearrange("p (t f) -> p t f", f=F),
                                    g_tab[:][BUCK * k:BUCK * (k + 1), :],
                                    idx_sb[:, k * TGK:(k + 1) * TGK],
                                    L, L, F, single_packet=False)
                                streams.append(stt)
                            for bb in range(GBLK):
                                B = g * GBLK + bb
                                agg = pp.tile([128, 128], f32, tag="agg")
                                first = True
                                for k in range(NBUCK):
                                    for t in range(TB):
                                        ti = bb * TB + t
                                        ind = wp.tile([128, 128], bf16, tag="ind")
                                        nc.vector.tensor_scalar(
                                            ind[:], iota_sb[:],
                                            ld_sb[:, k * TT + ti:k * TT + ti + 1],
                                            None, Alu.is_equal)
                                        nc.tensor.matmul(
                                            agg[:], ind[:],
                                            streams[k][:, ti * F:(ti + 1) * F],
                                            start=first, stop=False)
                                        first = False
                                gslf = wp.tile([128, 128], bf16, tag="gslf")
                                nc.sync.dma_start(
                                    out=gslf[:],
                                    in_=g_shard_self[:][B * 128:(B + 1) * 128, :])
                                nc.tensor.matmul(
                                    agg[:], ident_sb[:], gslf[:],
                                    start=False, stop=True)
                                h = wp.tile([128, 128], f32, tag="h")
                                nc.vector.scalar_tensor_tensor(
                                    h[:], agg[:], dis_sb[:, B:B + 1], b_sb[:],
                                    Alu.mult, Alu.add)
                                nc.scalar.activation(h[:], h[:], Act.Relu)
                                if layer == 1:
                                    hbf = wp.tile([128, 128], bf16, tag="hbf")
                                    nc.vector.tensor_copy(hbf[:], h[:])
                                    htp = pt.tile([128, 128], bf16, tag="htp")
                                    nc.tensor.transpose(htp[:], hbf[:], ident_sb[:])
                                    ht = wp.tile([128, 128], bf16, tag="ht")
                                    nc.scalar.activation(ht[:], htp[:], Act.Copy)
                                    t1 = pt.tile([128, 128], f32, tag="t1")
                                    nc.tensor.matmul(
                                        t1[:], ht[:], w2_sb[:], start=True, stop=True)
                                    g1b = wp.tile([128, 128], bf16, tag="g1b")
                                    nc.vector.tensor_scalar(
                                        g1b[:], t1[:], dis_sb[:, B:B + 1],
                                        None, Alu.mult)
                                    nc.sync.dma_start(
                                        out=out_shard[:][B * 128:(B + 1) * 128, :],
                                        in_=g1b[:])
                                else:
                                    ho = wp.tile([128, 132], bf16, tag="ho")
                                    nc.vector.tensor_copy(ho[:, :128], h[:])
                                    nc.vector.memset(ho[:, 128:132], 1.0)
                                    gi = wp.tile([128, 64], bf16, tag="gi")
                                    nc.vector.tensor_scalar(
                                        gi[:], iota_sb[:, :64], bat_sb[:, B:B + 1],
                                        None, Alu.is_equal)
                                    nc.tensor.matmul(
                                        pool_ps[:], gi[:], ho[:],
                                        start=(B == 0), stop=(B == NBLK - 1))

            edge_layer(1, g0_tab, g0_shard, g1_shard, b1_sb, None)
            nc.gpsimd.collective_compute(
                "AllGather", Alu.bypass,
                replica_groups=[list(range(NC))],
                ins=[g1_shard[:].opt()], outs=[g1_tab[:].opt()])

            with tc.tile_pool(name="pool_ps", bufs=1, space="PSUM") as plp:
                pool_ps = plp.tile([64, 132], f32)
                edge_layer(2, g1_tab, g1_shard, None, b2_sb, pool_ps)

                # ---- phase D: allreduce pooled sums, MLP head ----
                with tc.tile_pool(name="phd", bufs=1) as dp, \
                     tc.tile_pool(name="phd_ps", bufs=1, space="PSUM") as dps:
                    psum_sb = dp.tile([64, 132], f32)
                    nc.vector.tensor_copy(psum_sb[:], pool_ps[:])
                    pool_loc = dram.tile([64, 132], f32)
                    nc.sync.dma_start(out=pool_loc[:], in_=psum_sb[:])
                    pool_red = dram.tile([64, 132], f32)
                    nc.gpsimd.collective_compute(
                        "AllReduce", Alu.add,
                        replica_groups=[list(range(NC))],
                        ins=[pool_loc[:].opt()], outs=[pool_red[:].opt()])
                    red_sb = dp.tile([64, 132], f32)
                    nc.sync.dma_start(out=red_sb[:], in_=pool_red[:])

                    cnt = dp.tile([64, 1], f32)
                    nc.vector.tensor_scalar(
                        cnt[:], red_sb[:, 128:129], 1.0, None, Alu.max)
                    rcnt = dp.tile([64, 1], f32)
                    nc.vector.reciprocal(rcnt[:], cnt[:])
                    pooled = dp.tile([64, 128], f32)
                    nc.vector.tensor_scalar(
                        pooled[:], red_sb[:, :128], rcnt[:, 0:1], None, Alu.mult)

                    # z.T chunks
                    zt0p = dps.tile([128, 64], f32, tag="zt0p")
                    nc.tensor.matmul(zt0p[:], pooled[:], id64_sb[:],
                                     is_transpose=True, start=True, stop=True)
                    zt0 = dp.tile([128, 64], f32)
                    nc.scalar.activation(zt0[:], zt0p[:], Act.Copy)
                    zt1 = dp.tile([96, 64], f32)
                    nc.sync.dma_start(out=zt1[:64, :], in_=st_t[:, :])
                    nc.sync.dma_start(out=zt1[64:96, :], in_=ac_t[:, :])

                    wf1a = dp.tile([128, 256], f32)
                    nc.sync.dma_start(out=wf1a[:], in_=wf1[:][0:128, :])
                    wf1b = dp.tile([96, 256], f32)
                    nc.sync.dma_start(out=wf1b[:], in_=wf1[:][128:224, :])
                    wf2a = dp.tile([128, 256], f32)
                    nc.sync.dma_start(out=wf2a[:], in_=wf2[:][0:128, :])
                    wf2b = dp.tile([128, 256], f32)
                    nc.sync.dma_start(out=wf2b[:], in_=wf2[:][128:256, :])
                    wo_sb = dp.tile([128, 2], f32)
                    nc.sync.dma_start(out=wo_sb[:], in_=wo2[:, :])
                    bf1_sb = dp.tile([128, 2], f32)
                    nc.sync.dma_start(out=bf1_sb[:], in_=bf1[:, :])
                    bf2_sb = dp.tile([128, 2], f32)
                    nc.sync.dma_start(out=bf2_sb[:], in_=bf2[:, :])
                    bo_sb = dp.tile([1, 1], f32)
                    nc.sync.dma_start(out=bo_sb[:], in_=bo[:, :])

                    y1 = []
                    for m in range(2):
                        yp = dps.tile([128, 64], f32, tag=f"y1p{m}")
                        nc.tensor.matmul(yp[:], wf1a[:, m * 128:(m + 1) * 128],
                                         zt0[:], start=True, stop=False)
                        nc.tensor.matmul(yp[:], wf1b[:, m * 128:(m + 1) * 128],
                                         zt1[:], start=False, stop=True)
                        ys = dp.tile([128, 64], f32, tag=f"y1s{m}")
                        nc.scalar.activation(ys[:], yp[:], Act.Relu,
                                             bias=bf1_sb[:, m:m + 1], scale=1.0)
                        y1.append(ys)
                    y2 = []
                    for m in range(2):
                        yp = dps.tile([128, 64], f32, tag=f"y2p{m}")
                        nc.tensor.matmul(yp[:], wf2a[:, m * 128:(m + 1) * 128],
                                         y1[0][:], start=True, stop=False)
                        nc.tensor.matmul(yp[:], wf2b[:, m * 128:(m + 1) * 128],
                                         y1[1][:], start=False, stop=True)
                        ys = dp.tile([128, 64], f32, tag=f"y2s{m}")
                        nc.scalar.activation(ys[:], yp[:], Act.Relu,
                                             bias=bf2_sb[:, m:m + 1], scale=1.0)
                        y2.append(ys)
                    qp = dps.tile([1, 64], f32, tag="qp")
                    nc.tensor.matmul(qp[:], wo_sb[:, 0:1], y2[0][:],
                                     start=True, stop=False)
                    nc.tensor.matmul(qp[:], wo_sb[:, 1:2], y2[1][:],
                                     start=False, stop=True)
                    qs = dp.tile([1, 64], f32)
                    nc.scalar.activation(qs[:], qp[:], Act.Identity,
                                         bias=bo_sb[:, 0:1], scale=1.0)
                    nc.sync.dma_start(out=q_out[:, :], in_=qs[:])

    nc.compile()
    return nc


# ---------------- entry point ----------------

def _feeds_for(inputs, pre):
    iota_np = np.tile(np.arange(128, dtype=np.float32), (128, 1))
    feeds = []
    bf = ml_dtypes.bfloat16
    w1_np = np.asarray(inputs["W1"], np.float32)
    w2_np = np.asarray(inputs["W2"], np.float32)
    wf1_np = np.asarray(inputs["Wf1"], np.float32)
    wf2_np = np.asarray(inputs["Wf2"], np.float32)
    wo_np = np.asarray(inputs["Wo"], np.float32).reshape(256)
    b1_np = np.asarray(inputs["b1"], np.float32)
    b2_np = np.asarray(inputs["b2"], np.float32)
    bf1_np = np.asarray(inputs["bf1"], np.float32)
    bf2_np = np.asarray(inputs["bf2"], np.float32)
    bo_np = np.asarray(inputs["bo"], np.float32).reshape(1, 1)
    st_np = np.asarray(inputs["state_vector"], np.float32)
    ac_np = np.asarray(inputs["action"], np.float32)
    for c in range(NC):
        feeds.append(dict(
            xt=pre["xt_feed"][c],
            deg=pre["deg_feed"][c],
            bat=pre["bat_feed"][c],
            idxs=pre["idx_feed"][c],
            lds=pre["ld_feed"][c],
            iota=iota_np.astype(bf),
            ident=np.eye(128, dtype=np.float32).astype(bf),
            id64=np.eye(64, dtype=np.float32),
            b1t=np.tile(b1_np, (128, 1)),
            b2t=np.tile(b2_np, (128, 1)),
            w1=w1_np, w2=w2_np, wf1=wf1_np, wf2=wf2_np,
            wo2=np.stack([wo_np[:128], wo_np[128:]], axis=1),
            bf1=np.stack([bf1_np[:128], bf1_np[128:]], axis=1),
            bf2=np.stack([bf2_np[:128], bf2_np[128:]], axis=1),
            bo=bo_np,
            st_t=np.ascontiguousarray(st_np.T),
            ac_t=np.ascontiguousarray(ac_np.T),
        ))
    return feeds


def kernel(**inputs):
    global LAST_RESULT
    from concourse.bass_utils import run_bass_kernel_spmd
    import concourse.bass_utils as bu

    _install_ntff_hook()
    bu.upload_artifacts = lambda d: d

    pre = _preprocess(inputs)
    S = pre["S"]
    if S not in _PROGRAM_CACHE:
        _PROGRAM_CACHE[S] = _build_program(S)
    nc = _PROGRAM_CACHE[S]
    feeds = _feeds_for(inputs, pre)
    res = run_bass_kernel_spmd(nc, feeds, core_ids=list(range(NC)))
    LAST_RESULT = res
    q = np.asarray(res.results[0]["q_out"], np.float32).reshape(64, 1)
    return q
